# revision 6
# baseline (speedup 1.0000x reference)
"""RBF Gram matrix kernel for Trainium2, 8-core SPMD.

K[i, j] = exp(-gamma * ||x_i - s_j||^2),  x [8192, 256] f32, support [8192, 256] f32.

Strategy (v3):
  - 4x2 shard grid: x rows split into 4 strips of 2048, support cols into 2
    halves of 4096. Core (r, h) computes the [2048, 4096] block.
  - exp(-g*||x-s||^2) = exp(2g*x.s - g*||x||^2) * exp(-g*||s||^2).
    GEMM computes x.s only (2 chunks of K=128, fp16); the row term rides the
    ScalarE activation as a per-partition bias; the column factor is one fp16
    VectorE tensor_tensor multiply against a precomputed broadcast tile.
  - Output written fp16 (halves store traffic vs f32); host upcasts.
  - Startup: operand tiles are split small and loaded over three DMA queues in
    first-use order; the first m-tile runs 512-wide PSUM groups so ScalarE
    starts early. Stores go out in 2048-wide halves to shorten the tail.
"""

import numpy as np

try:
    import concourse.bass as bass  # noqa: F401
except ImportError:
    import sys

    sys.path.insert(0, "/opt/trn_rl_repo")

N, M, D = 8192, 8192, 256
GAMMA = 1.0 / D
NCORES = 8
RSH, CSH = 4, 2  # row shards x col shards
SR = N // RSH  # 2048 x-rows per core
SC = M // CSH  # 4096 support-cols per core
P = 128
NTILE = 512  # matmul free-dim slice
NGROUP = 2048  # PSUM group: 4 banks, one ACTIVATE + one DVE mult per group
XSUB = 1024  # xa sub-tile width
SSUB = 1024  # sa sub-tile width

_CACHE = {}


def _build():
    import concourse.tile as tile
    from concourse import bacc, mybir

    f16 = mybir.dt.float16
    f32 = mybir.dt.float32

    nc = bacc.Bacc("TRN2", target_bir_lowering=False, debug=False, num_devices=NCORES)

    xa = nc.dram_tensor("xa", [2, P, SR], f16, kind="ExternalInput")
    sa = nc.dram_tensor("sa", [2, P, SC], f16, kind="ExternalInput")
    cb = nc.dram_tensor("cb", [P, SC], f16, kind="ExternalInput")
    xb = nc.dram_tensor("xb", [P, SR // P], f32, kind="ExternalInput")
    out = nc.dram_tensor("out", [SR // P, P, SC], f16, kind="ExternalOutput")

    n_mt = SR // P  # 16 m-tiles
    n_xs = SR // XSUB  # 2 xa sub-tiles per chunk
    n_ss = SC // SSUB  # 4 sa sub-tiles per chunk

    with tile.TileContext(nc) as tc:
        with (
            tc.tile_pool(name="const", bufs=1) as const,
            tc.tile_pool(name="psum", bufs=2, space="PSUM") as psum_pool,
            tc.tile_pool(name="ebuf", bufs=4) as ebuf,
            tc.tile_pool(name="obuf", bufs=3) as obuf,
        ):
            # --- operand tiles, loaded in first-use order over 3 queues ---
            xb_t = const.tile([P, SR // P], f32, tag="xb")
            xa_t = [
                [const.tile([P, XSUB], f16, name=f"xa{c}_{j}", tag=f"xa{c}_{j}") for j in range(n_xs)]
                for c in range(2)
            ]
            sa_t = [
                [const.tile([P, SSUB], f16, name=f"sa{c}_{j}", tag=f"sa{c}_{j}") for j in range(n_ss)]
                for c in range(2)
            ]
            cb_t = [const.tile([P, NGROUP], f16, name=f"cb{g}", tag=f"cb{g}") for g in range(2)]

            # queue 1 (SP): bias + x chunks for the first m-tiles
            nc.sync.dma_start(out=xb_t[:], in_=xb[:])
            nc.sync.dma_start(out=xa_t[0][0][:], in_=xa[0, :, 0:XSUB])
            nc.sync.dma_start(out=xa_t[1][0][:], in_=xa[1, :, 0:XSUB])
            # queue 2 (Activation): first col-group of support + first cb half
            nc.scalar.dma_start(out=sa_t[0][0][:], in_=sa[0, :, 0:SSUB])
            nc.scalar.dma_start(out=sa_t[1][0][:], in_=sa[1, :, 0:SSUB])
            nc.scalar.dma_start(out=sa_t[0][1][:], in_=sa[0, :, SSUB : 2 * SSUB])
            nc.scalar.dma_start(out=sa_t[1][1][:], in_=sa[1, :, SSUB : 2 * SSUB])
            nc.scalar.dma_start(out=cb_t[0][:], in_=cb[:, 0:NGROUP])
            # queue 3 (Pool/SWDGE): second col-group + stragglers
            nc.gpsimd.dma_start(out=sa_t[0][2][:], in_=sa[0, :, 2 * SSUB : 3 * SSUB])
            nc.gpsimd.dma_start(out=sa_t[1][2][:], in_=sa[1, :, 2 * SSUB : 3 * SSUB])
            nc.gpsimd.dma_start(out=sa_t[0][3][:], in_=sa[0, :, 3 * SSUB : 4 * SSUB])
            nc.gpsimd.dma_start(out=sa_t[1][3][:], in_=sa[1, :, 3 * SSUB : 4 * SSUB])
            nc.gpsimd.dma_start(out=cb_t[1][:], in_=cb[:, NGROUP : 2 * NGROUP])
            nc.sync.dma_start(out=xa_t[0][1][:], in_=xa[0, :, XSUB : 2 * XSUB])
            nc.sync.dma_start(out=xa_t[1][1][:], in_=xa[1, :, XSUB : 2 * XSUB])

            def sa_slice(c, n):
                """[128, NTILE] slice n of chunk c from the sa sub-tiles."""
                j, r = divmod(n * NTILE, SSUB)
                return sa_t[c][j][:, r : r + NTILE]

            def do_group(m, g, width, ot):
                """One PSUM group: matmuls + ACT exp + DVE col-multiply."""
                ms = slice((m * P) % XSUB, (m * P) % XSUB + P)
                jx = (m * P) // XSUB
                ps = psum_pool.tile([P, width], f32)
                for c in range(2):
                    lhsT = xa_t[c][jx][:, ms]
                    for k in range(width // NTILE):
                        n = (g * width) // NTILE + k
                        nc.tensor.matmul(
                            ps[:, k * NTILE : (k + 1) * NTILE],
                            lhsT,
                            sa_slice(c, n),
                            start=(c == 0),
                            stop=(c == 1),
                        )
                et = ebuf.tile([P, width], f16)
                nc.scalar.activation(
                    et[:],
                    ps[:],
                    mybir.ActivationFunctionType.Exp,
                    bias=xb_t[:, m : m + 1],
                    scale=2.0 * GAMMA,
                )
                gs = slice(g * width, (g + 1) * width)
                cg, cr = divmod(g * width, NGROUP)
                nc.vector.tensor_tensor(
                    out=ot[:, gs],
                    in0=et[:],
                    in1=cb_t[cg][:, cr : cr + width],
                    op=mybir.AluOpType.mult,
                )
                return gs

            for m in range(n_mt):
                ot = obuf.tile([P, SC], f16)
                if m == 0:
                    # fine-grained first tile: ScalarE starts ASAP
                    for g in range(SC // NTILE):
                        gs = do_group(m, g, NTILE, ot)
                        if g % 4 == 3:
                            hs = slice(g * NTILE - 3 * NTILE, (g + 1) * NTILE)
                            nc.sync.dma_start(out=out[m][:, hs], in_=ot[:, hs])
                else:
                    for g in range(SC // NGROUP):
                        gs = do_group(m, g, NGROUP, ot)
                        nc.sync.dma_start(out=out[m][:, gs], in_=ot[:, gs])
    nc.compile()
    return nc


def kernel(x, support):
    from concourse.bass_utils import run_bass_kernel_spmd

    if "nc" not in _CACHE:
        _CACHE["nc"] = _build()
    nc = _CACHE["nc"]

    x = np.asarray(x, dtype=np.float32)
    support = np.asarray(support, dtype=np.float32)

    x_sq = np.einsum("nd,nd->n", x, x)
    s_sq = np.einsum("md,md->m", support, support)

    # [256, N] fp16, contraction on rows; split into 2 chunks of 128
    xT = np.ascontiguousarray(x.T.astype(np.float16)).reshape(2, P, N)
    sT = np.ascontiguousarray(support.T.astype(np.float16)).reshape(2, P, M)

    col_factor = np.exp(-GAMMA * s_sq).astype(np.float16)  # [M]
    row_bias = (-GAMMA * x_sq).astype(np.float32)  # [N]

    xa_r = [np.ascontiguousarray(xT[:, :, r * SR : (r + 1) * SR]) for r in range(RSH)]
    xb_r = [
        np.ascontiguousarray(row_bias[r * SR : (r + 1) * SR].reshape(SR // P, P).T)
        for r in range(RSH)
    ]
    sa_h = [np.ascontiguousarray(sT[:, :, h * SC : (h + 1) * SC]) for h in range(CSH)]
    cb_h = [
        np.ascontiguousarray(
            np.broadcast_to(col_factor[h * SC : (h + 1) * SC], (P, SC))
        )
        for h in range(CSH)
    ]

    in_maps = []
    for r in range(RSH):
        for h in range(CSH):
            in_maps.append({"xa": xa_r[r], "sa": sa_h[h], "cb": cb_h[h], "xb": xb_r[r]})

    res = run_bass_kernel_spmd(nc, in_maps, list(range(NCORES)))

    final = np.empty((N, M), dtype=np.float32)
    for r in range(RSH):
        for h in range(CSH):
            piece = res.results[r * CSH + h]["out"]  # [16, 128, SC] f16
            final[r * SR : (r + 1) * SR, h * SC : (h + 1) * SC] = piece.reshape(
                SR, SC
            ).astype(np.float32)
    return final


# revision 7
# speedup vs baseline: 1.0235x; 1.0235x over previous
"""RBF Gram matrix kernel for Trainium2, 8-core SPMD.

K[i, j] = exp(-gamma * ||x_i - s_j||^2),  x [8192, 256] f32, support [8192, 256] f32.

Strategy (v3):
  - 4x2 shard grid: x rows split into 4 strips of 2048, support cols into 2
    halves of 4096. Core (r, h) computes the [2048, 4096] block.
  - exp(-g*||x-s||^2) = exp(2g*x.s - g*||x||^2) * exp(-g*||s||^2).
    GEMM computes x.s only (2 chunks of K=128, fp16); the row term rides the
    ScalarE activation as a per-partition bias; the column factor is one fp16
    VectorE tensor_tensor multiply against a precomputed broadcast tile.
  - Output written fp16 (halves store traffic vs f32); host upcasts.
  - Startup: operand tiles are split small and loaded over three DMA queues in
    first-use order; the first m-tile runs 512-wide PSUM groups so ScalarE
    starts early. Stores go out in 2048-wide halves to shorten the tail.
"""

import numpy as np

try:
    import concourse.bass as bass  # noqa: F401
except ImportError:
    import sys

    sys.path.insert(0, "/opt/trn_rl_repo")

N, M, D = 8192, 8192, 256
GAMMA = 1.0 / D
NCORES = 8
RSH, CSH = 4, 2  # row shards x col shards
SR = N // RSH  # 2048 x-rows per core
SC = M // CSH  # 4096 support-cols per core
P = 128
NTILE = 512  # matmul free-dim slice
NGROUP = 2048  # PSUM group: 4 banks, one ACTIVATE + one DVE mult per group
XSUB = 1024  # xa sub-tile width
SSUB = 1024  # sa sub-tile width

_CACHE = {}


def _build():
    import concourse.tile as tile
    from concourse import bacc, mybir

    f16 = mybir.dt.float16
    f32 = mybir.dt.float32

    nc = bacc.Bacc("TRN2", target_bir_lowering=False, debug=False, num_devices=NCORES)

    xa = nc.dram_tensor("xa", [2, P, SR], f16, kind="ExternalInput")
    sa = nc.dram_tensor("sa", [2, P, SC], f16, kind="ExternalInput")
    cb = nc.dram_tensor("cb", [P, SC], f16, kind="ExternalInput")
    xb = nc.dram_tensor("xb", [P, SR // P], f32, kind="ExternalInput")
    out = nc.dram_tensor("out", [SR // P, P, SC], f16, kind="ExternalOutput")

    n_mt = SR // P  # 16 m-tiles
    n_xs = SR // XSUB  # 2 xa sub-tiles per chunk
    n_ss = SC // SSUB  # 4 sa sub-tiles per chunk

    with tile.TileContext(nc) as tc:
        with (
            tc.tile_pool(name="const", bufs=1) as const,
            tc.tile_pool(name="psum", bufs=2, space="PSUM") as psum_pool,
            tc.tile_pool(name="ebuf", bufs=4) as ebuf,
            tc.tile_pool(name="obuf", bufs=3) as obuf,
        ):
            # --- operand tiles, loaded in first-use order over 3 queues ---
            xb_t = const.tile([P, SR // P], f32, tag="xb")
            xa_t = [
                [const.tile([P, XSUB], f16, name=f"xa{c}_{j}", tag=f"xa{c}_{j}") for j in range(n_xs)]
                for c in range(2)
            ]
            sa_t = [
                [const.tile([P, SSUB], f16, name=f"sa{c}_{j}", tag=f"sa{c}_{j}") for j in range(n_ss)]
                for c in range(2)
            ]
            cb_t = [const.tile([P, NGROUP], f16, name=f"cb{g}", tag=f"cb{g}") for g in range(2)]

            # Loads: each queue's internal order matches first-use order.
            # queue 1 (SP): bias + x chunks
            nc.sync.dma_start(out=xb_t[:], in_=xb[:])
            nc.sync.dma_start(out=xa_t[0][0][:], in_=xa[0, :, 0:XSUB])
            nc.sync.dma_start(out=xa_t[1][0][:], in_=xa[1, :, 0:XSUB])
            nc.sync.dma_start(out=xa_t[0][1][:], in_=xa[0, :, XSUB : 2 * XSUB])
            nc.sync.dma_start(out=xa_t[1][1][:], in_=xa[1, :, XSUB : 2 * XSUB])
            # queue 2 (Activation): first support col-group + its col factors
            nc.scalar.dma_start(out=sa_t[0][0][:], in_=sa[0, :, 0:SSUB])
            nc.scalar.dma_start(out=sa_t[1][0][:], in_=sa[1, :, 0:SSUB])
            nc.scalar.dma_start(out=cb_t[0][:], in_=cb[:, 0:NGROUP])
            # queue 3 (Pool/SWDGE): the rest, in need order
            nc.gpsimd.dma_start(out=sa_t[0][1][:], in_=sa[0, :, SSUB : 2 * SSUB])
            nc.gpsimd.dma_start(out=sa_t[1][1][:], in_=sa[1, :, SSUB : 2 * SSUB])
            nc.gpsimd.dma_start(out=cb_t[1][:], in_=cb[:, NGROUP : 2 * NGROUP])
            nc.gpsimd.dma_start(out=sa_t[0][2][:], in_=sa[0, :, 2 * SSUB : 3 * SSUB])
            nc.gpsimd.dma_start(out=sa_t[1][2][:], in_=sa[1, :, 2 * SSUB : 3 * SSUB])
            nc.gpsimd.dma_start(out=sa_t[0][3][:], in_=sa[0, :, 3 * SSUB : 4 * SSUB])
            nc.gpsimd.dma_start(out=sa_t[1][3][:], in_=sa[1, :, 3 * SSUB : 4 * SSUB])

            def sa_slice(c, n):
                """[128, NTILE] slice n of chunk c from the sa sub-tiles."""
                j, r = divmod(n * NTILE, SSUB)
                return sa_t[c][j][:, r : r + NTILE]

            def do_group(m, g, width, ot):
                """One PSUM group: matmuls + ACT exp + DVE col-multiply."""
                ms = slice((m * P) % XSUB, (m * P) % XSUB + P)
                jx = (m * P) // XSUB
                ps = psum_pool.tile([P, width], f32)
                for c in range(2):
                    lhsT = xa_t[c][jx][:, ms]
                    for k in range(width // NTILE):
                        n = (g * width) // NTILE + k
                        nc.tensor.matmul(
                            ps[:, k * NTILE : (k + 1) * NTILE],
                            lhsT,
                            sa_slice(c, n),
                            start=(c == 0),
                            stop=(c == 1),
                        )
                et = ebuf.tile([P, width], f16)
                nc.scalar.activation(
                    et[:],
                    ps[:],
                    mybir.ActivationFunctionType.Exp,
                    bias=xb_t[:, m : m + 1],
                    scale=2.0 * GAMMA,
                )
                gs = slice(g * width, (g + 1) * width)
                cg, cr = divmod(g * width, NGROUP)
                nc.vector.tensor_tensor(
                    out=ot[:, gs],
                    in0=et[:],
                    in1=cb_t[cg][:, cr : cr + width],
                    op=mybir.AluOpType.mult,
                )
                return gs

            for m in range(n_mt):
                ot = obuf.tile([P, SC], f16)
                if m == 0:
                    # fine-grained first tile: ScalarE starts ASAP
                    for g in range(SC // NTILE):
                        gs = do_group(m, g, NTILE, ot)
                        if g % 4 == 3:
                            hs = slice(g * NTILE - 3 * NTILE, (g + 1) * NTILE)
                            nc.sync.dma_start(out=out[m][:, hs], in_=ot[:, hs])
                else:
                    for g in range(SC // NGROUP):
                        gs = do_group(m, g, NGROUP, ot)
                        nc.sync.dma_start(out=out[m][:, gs], in_=ot[:, gs])
    nc.compile()
    return nc


def kernel(x, support):
    from concourse.bass_utils import run_bass_kernel_spmd

    if "nc" not in _CACHE:
        _CACHE["nc"] = _build()
    nc = _CACHE["nc"]

    x = np.asarray(x, dtype=np.float32)
    support = np.asarray(support, dtype=np.float32)

    x_sq = np.einsum("nd,nd->n", x, x)
    s_sq = np.einsum("md,md->m", support, support)

    # [256, N] fp16, contraction on rows; split into 2 chunks of 128
    xT = np.ascontiguousarray(x.T.astype(np.float16)).reshape(2, P, N)
    sT = np.ascontiguousarray(support.T.astype(np.float16)).reshape(2, P, M)

    col_factor = np.exp(-GAMMA * s_sq).astype(np.float16)  # [M]
    row_bias = (-GAMMA * x_sq).astype(np.float32)  # [N]

    xa_r = [np.ascontiguousarray(xT[:, :, r * SR : (r + 1) * SR]) for r in range(RSH)]
    xb_r = [
        np.ascontiguousarray(row_bias[r * SR : (r + 1) * SR].reshape(SR // P, P).T)
        for r in range(RSH)
    ]
    sa_h = [np.ascontiguousarray(sT[:, :, h * SC : (h + 1) * SC]) for h in range(CSH)]
    cb_h = [
        np.ascontiguousarray(
            np.broadcast_to(col_factor[h * SC : (h + 1) * SC], (P, SC))
        )
        for h in range(CSH)
    ]

    in_maps = []
    for r in range(RSH):
        for h in range(CSH):
            in_maps.append({"xa": xa_r[r], "sa": sa_h[h], "cb": cb_h[h], "xb": xb_r[r]})

    res = run_bass_kernel_spmd(nc, in_maps, list(range(NCORES)))

    final = np.empty((N, M), dtype=np.float32)
    for r in range(RSH):
        for h in range(CSH):
            piece = res.results[r * CSH + h]["out"]  # [16, 128, SC] f16
            final[r * SR : (r + 1) * SR, h * SC : (h + 1) * SC] = piece.reshape(
                SR, SC
            ).astype(np.float32)
    return final
